# revision 20
# baseline (speedup 1.0000x reference)
"""Multi-head attention (B=2, N=2048, C=1024, H=16, D=64) on 8 TRN2 NeuronCores.

Sharding: 2-way data parallel on batch x 4-way tensor parallel on heads
(4 heads per core). w_qkv column-sharded, w_proj row-sharded; the final
all-reduce over head groups is done host-side (sum of 4 partial outputs
per batch) along with the b_proj add.

Per-core device kernel (all matmul compute in bf16, f32 accumulation):
  phase 1: qk^T = wqk^T @ x^T   -> q^T,k^T [512, 2048] (head-major rows)
           v    = (x^T)^T @ wv  -> v [2048, 256] natural, + ones column
  phase 2: per (i-block, head): scores^T = k^T(slice)^T-free matmuls
           (K=64 contraction), exp on ScalarE (PSUM->SBUF bf16),
           attn@v with v augmented by a ones column (M=65) so row 64
           accumulates the softmax denominator; normalize via
           reciprocal + gpsimd partition_broadcast + DVE multiply.
  phase 3: y = outT^T @ wp  [2048, 1024] f32 partial, DMA out.
"""

import numpy as np
import ml_dtypes

import concourse.bacc as bacc
import concourse.mybir as mybir
import concourse.tile as tile
from concourse.bass_utils import run_bass_kernel_spmd

BF = mybir.dt.bfloat16
F32 = mybir.dt.float32

B = 2
N = 2048
C = 1024
HEADS = 16
D = 64
SCALE = D ** -0.5
G = 4            # head groups (tensor-parallel degree)
HPG = 4          # heads per group (local heads per core)
DL = HPG * D     # 256 local inner dim
NQK = 2 * DL     # 512 local q+k columns
KC = C // 128    # 8 contraction chunks
NB = N // 128    # 16 n-blocks
IBS = 512        # i-block size
IB = N // IBS    # 4 i-blocks
JBL = N // 128   # 16 j-blocks

EXPJ = 2         # j-blocks per exp op (psum scores tile = [128, EXPJ, 512])


def build_graph(expj=EXPJ, loop_r=None, exp_half=False):
    import contextlib

    nc = bacc.Bacc("TRN2")
    xT_ext = nc.declare_dram_parameter("xT", [C, N], BF, isOutput=False)
    wqk_ext = nc.declare_dram_parameter("wqk", [C, NQK], BF, isOutput=False)
    wv_ext = nc.declare_dram_parameter("wv", [C, DL], BF, isOutput=False)
    wp_ext = nc.declare_dram_parameter("wp", [DL, C], BF, isOutput=False)
    y_ext = nc.declare_dram_parameter("y", [N, C], F32, isOutput=True)

    Exp = mybir.ActivationFunctionType.Exp

    with tile.TileContext(nc) as tc:
        with (
            tc.tile_pool(name="const", bufs=1) as cpool,
            tc.tile_pool(name="probs", bufs=3) as probs_pool,
            tc.tile_pool(name="veca", bufs=2) as veca_pool,
            tc.tile_pool(name="recip1", bufs=1) as recip_pool,
            tc.tile_pool(name="obuf", bufs=2) as ob_pool,
            tc.tile_pool(name="yout", bufs=2) as y_pool,
            (tc.For_i(0, loop_r, 1) if loop_r else contextlib.nullcontext()),
        ):
            xT_sb = cpool.tile([128, KC, N], BF, tag="xT")
            wqk_sb = cpool.tile([128, KC, NQK], BF, tag="wqk")
            wv_sb = cpool.tile([128, KC, DL], BF, tag="wv")
            wp_sb = cpool.tile([128, 2, C], BF, tag="wp")
            qkT_sb = cpool.tile([128, 4, N], BF, tag="qkT")
            v_sb = cpool.tile([128, NB, HPG, D + 1], BF, tag="v")
            outT_sb = cpool.tile([128, 2, N], BF, tag="outT")

            for k in range(KC):
                nc.sync.dma_start(xT_sb[:, k, :], xT_ext[k * 128:(k + 1) * 128, :])
                nc.sync.dma_start(wqk_sb[:, k, :], wqk_ext[k * 128:(k + 1) * 128, :])
                nc.sync.dma_start(wv_sb[:, k, :], wv_ext[k * 128:(k + 1) * 128, :])
            for kc in range(2):
                nc.sync.dma_start(wp_sb[:, kc, :], wp_ext[kc * 128:(kc + 1) * 128, :])

            # ones column for the v-augmented denominator trick
            for nb in range(NB):
                nc.vector.memset(v_sb[:, nb, :, D:D + 1], 1.0)

            ss_sizes = []
            j = 0
            while j < JBL:
                n = min(expj, JBL - j)
                ss_sizes.append((j, n))
                j += n

            with (
                tc.tile_pool(name="ps_qk", bufs=2, space="PSUM") as psA,
                tc.tile_pool(name="ps_s", bufs=2, space="PSUM") as pss_pool,
                tc.tile_pool(name="ps_o", bufs=2, space="PSUM") as pso_pool,
            ):
                def emit_qk(mi, blocks=range(4)):
                    # qk^T: out [m-block 128, n 512] = wqk_slice^T @ xT
                    for nb4 in blocks:
                        ps = psA.tile([128, 512], F32, tag="psqk")
                        for k in range(KC):
                            nc.tensor.matmul(
                                ps[:],
                                wqk_sb[:, k, mi * 128:(mi + 1) * 128],
                                xT_sb[:, k, nb4 * 512:(nb4 + 1) * 512],
                                start=(k == 0),
                                stop=(k == KC - 1),
                            )
                        nc.vector.tensor_copy(
                            qkT_sb[:, mi, nb4 * 512:(nb4 + 1) * 512], ps[:]
                        )

                def emit_v():
                    # v natural: out [n-block 128, 256] = xT_slice^T @ wv
                    for nb in range(NB):
                        psv = psA.tile([128, DL], F32, tag="psqk")
                        for k in range(KC):
                            nc.tensor.matmul(
                                psv[:],
                                xT_sb[:, k, nb * 128:(nb + 1) * 128],
                                wv_sb[:, k, :],
                                start=(k == 0),
                                stop=(k == KC - 1),
                            )
                        nc.vector.tensor_copy(
                            v_sb[:, nb, :, 0:D],
                            psv[:].rearrange("p (h d) -> p h d", d=D),
                        )

                probs_tiles = {}

                def emit_scores_superstep(ib, hp, jb):
                    # heads (2hp, 2hp+1): row-group-paired score matmuls.
                    # Head parity selects PE row groups (lhsT partition base 0
                    # vs 64), so adjacent per-head matmuls run concurrently.
                    i0 = ib * IBS
                    probs = probs_tiles[(ib, hp)]
                    pss = pss_pool.tile([128, 2, IBS], F32, tag="pss")
                    for hs in range(2):
                        hb = hs * 64
                        nc.tensor.matmul(
                            pss[:, hs, :],
                            qkT_sb[hb:hb + 64, 2 + hp,
                                   jb * 128:(jb + 1) * 128],
                            qkT_sb[hb:hb + 64, hp, i0:i0 + IBS],
                            start=True,
                            stop=True,
                        )
                    if not exp_half or jb % 2 == 0:
                        nc.scalar.activation(probs[:, jb, :, :], pss[:], Exp)

                def alloc_probs(ib, hp):
                    probs_t = probs_pool.tile([128, JBL, 2, IBS], BF, tag="probs")
                    probs_tiles[(ib, hp)] = probs_t

                def emit_scores_pair(ib, hp):
                    alloc_probs(ib, hp)
                    for jb in range(JBL):
                        emit_scores_superstep(ib, hp, jb)

                def emit_norm(ib, h, pso):
                    # outT[hb:hb+64, hp, i] = pso[0:64] / pso[64]. Copy the
                    # PSUM accumulator to SBUF first so its bank frees for the
                    # next unit's av matmuls ~3us earlier than the full
                    # recip/broadcast/mul chain would allow.
                    i0 = ib * IBS
                    hb = (h % 2) * 64
                    hp = h // 2
                    ob = ob_pool.tile([65, IBS], F32, tag="ob")
                    nc.vector.tensor_copy(ob[:], pso[:])
                    recip = recip_pool.tile([1, IBS], F32, tag="recip")
                    nc.vector.reciprocal(recip[0:1, :], ob[64:65, :])
                    rep = veca_pool.tile([64, IBS], F32, tag="rep")
                    nc.gpsimd.partition_broadcast(rep[:], recip[0:1, :])
                    nc.vector.tensor_mul(
                        outT_sb[hb:hb + 64, hp, i0:i0 + IBS],
                        ob[0:64, :],
                        rep[:],
                    )

                ysb_tiles = {}

                def emit_proj_piece(nb, cb):
                    if cb == 0:
                        ysb_t = y_pool.tile([128, C], F32, tag="ysb")
                        ysb_tiles[nb] = ysb_t
                    ysb = ysb_tiles[nb]
                    psy = psA.tile([128, 512], F32, tag="psqk")
                    for kc in range(2):
                        nc.tensor.matmul(
                            psy[:],
                            outT_sb[:, kc, nb * 128:(nb + 1) * 128],
                            wp_sb[:, kc, cb * 512:(cb + 1) * 512],
                            start=(kc == 0),
                            stop=(kc == 1),
                        )
                    nc.vector.tensor_copy(ysb[:, cb * 512:(cb + 1) * 512], psy[:])
                    if cb == 1:
                        nc.sync.dma_start(y_ext[nb * 128:(nb + 1) * 128, :], ysb[:])
                        ysb_tiles.pop(nb)

                # phase 1 interleaved with the start of attention: compute k01
                # and q01 first so head 0/1 softmax starts ~35us earlier; v and
                # the other head-pair's qk fill the PE while ACT churns exps.
                # ramp: score superstep jb only needs k column-block jb//4,
                # so interleave the k01 blocks with the first unit's scores —
                # the exp stream starts after ~2 qk blocks instead of 5.
                emit_qk(2, blocks=(0,))  # k heads 0,1, j-blocks 0-3
                emit_qk(0, blocks=(0,))  # q heads 0,1, i-block 0
                alloc_probs(0, 0)
                for jb in range(4):
                    emit_scores_superstep(0, 0, jb)
                for nb4 in (1, 2, 3):
                    emit_qk(2, blocks=(nb4,))
                    for jb in range(4 * nb4, 4 * nb4 + 4):
                        emit_scores_superstep(0, 0, jb)
                emit_qk(0, blocks=(1, 2, 3))
                emit_qk(3)              # k heads 2,3
                emit_qk(1)              # q heads 2,3
                emit_scores_pair(0, 1)
                emit_v()
                # steady state, interleaved at j-block granularity so ScalarE
                # always has a fresh score superstep to exp: per unit u emit
                # [scores(u+2, jb), av(u, even, jb), av(u, odd, jb), proj piece]
                units = [(ib, hp) for ib in range(IB) for hp in range(2)]
                pending_proj = []
                for idx, (ib, hp) in enumerate(units):
                    nxt = units[idx + 2] if idx + 2 < len(units) else None
                    if nxt is not None:
                        probs_n = probs_pool.tile([128, JBL, 2, IBS], BF, tag="probs")
                        probs_tiles[nxt] = probs_n
                    probs_c = probs_tiles.pop((ib, hp))
                    pso_e = pso_pool.tile([65, IBS], F32, tag="pso")
                    pso_o = pso_pool.tile([65, IBS], F32, tag="pso")
                    for jb in range(JBL):
                        if nxt is not None:
                            emit_scores_superstep(nxt[0], nxt[1], jb)
                        nc.tensor.matmul(
                            pso_e[:], v_sb[:, jb, 2 * hp, :],
                            probs_c[:, jb, 0, :],
                            start=(jb == 0), stop=(jb == JBL - 1),
                        )
                        nc.tensor.matmul(
                            pso_o[:], v_sb[:, jb, 2 * hp + 1, :],
                            probs_c[:, jb, 1, :],
                            start=(jb == 0), stop=(jb == JBL - 1),
                        )
                        if pending_proj and jb % 2 == 1:
                            emit_proj_piece(*pending_proj.pop(0))
                    emit_norm(ib, 2 * hp, pso_e)
                    emit_norm(ib, 2 * hp + 1, pso_o)
                    if hp == 1:
                        pending_proj.extend(
                            (nb, cb)
                            for nb in range(ib * 4, ib * 4 + 4)
                            for cb in range(2)
                        )
                while pending_proj:
                    emit_proj_piece(*pending_proj.pop(0))

    nc.finalize()
    return nc


_GRAPH = None


def _get_graph():
    global _GRAPH
    if _GRAPH is None:
        _GRAPH = build_graph()
    return _GRAPH


def make_in_maps(x, w_qkv, w_proj):
    bf = ml_dtypes.bfloat16
    in_maps = []
    for core in range(8):
        b = core // G
        g = core % G
        wq = w_qkv[:, g * DL:(g + 1) * DL] * SCALE
        wk = w_qkv[:, C + g * DL:C + (g + 1) * DL]
        in_maps.append({
            "xT": np.ascontiguousarray(x[b].T).astype(bf),
            "wqk": np.concatenate([wq, wk], axis=1).astype(bf),
            "wv": np.ascontiguousarray(w_qkv[:, 2 * C + g * DL:2 * C + (g + 1) * DL]).astype(bf),
            "wp": np.ascontiguousarray(w_proj[g * DL:(g + 1) * DL, :]).astype(bf),
        })
    return in_maps


def gather_output(results, b_proj):
    y = np.zeros((B, N, C), np.float32)
    for core in range(8):
        y[core // G] += np.asarray(results[core]["y"], np.float32)
    y += np.asarray(b_proj, np.float32)[None, None, :]
    return y


def kernel(x, w_qkv, w_proj, b_proj):
    import time as _time

    nc = _get_graph()
    in_maps = make_in_maps(np.asarray(x, np.float32), np.asarray(w_qkv, np.float32),
                           np.asarray(w_proj, np.float32))
    last_err = None
    for attempt in range(3):
        try:
            res = run_bass_kernel_spmd(nc, in_maps, core_ids=list(range(8)))
            return gather_output(res.results, np.asarray(b_proj, np.float32))
        except Exception as e:  # transient NRT device errors: retry
            last_err = e
            _time.sleep(5.0 * (attempt + 1))
    raise last_err


# revision 22
# speedup vs baseline: 1.4674x; 1.4674x over previous
"""Multi-head attention (B=2, N=2048, C=1024, H=16, D=64) on 8 TRN2 NeuronCores.

Sharding: 2-way data parallel on batch x 4-way tensor parallel on heads
(4 heads per core). w_qkv column-sharded, w_proj row-sharded; the final
all-reduce over head groups is done host-side (sum of 4 partial outputs
per batch) along with the b_proj add.

Per-core device kernel (all matmul compute in bf16, f32 accumulation):
  phase 1: qk^T = wqk^T @ x^T   -> q^T,k^T [512, 2048] (head-major rows)
           v    = (x^T)^T @ wv  -> v [2048, 256] natural, + ones column
  phase 2: per (i-block, head): scores^T = k^T(slice)^T-free matmuls
           (K=64 contraction), exp on ScalarE (PSUM->SBUF bf16),
           attn@v with v augmented by a ones column (M=65) so row 64
           accumulates the softmax denominator; normalize via
           reciprocal + gpsimd partition_broadcast + DVE multiply.
  phase 3: y = outT^T @ wp  [2048, 1024] f32 partial, DMA out.
"""

import numpy as np
import ml_dtypes

import concourse.bacc as bacc
import concourse.mybir as mybir
import concourse.tile as tile
from concourse.bass_utils import run_bass_kernel_spmd

BF = mybir.dt.bfloat16
F32 = mybir.dt.float32

B = 2
N = 2048
C = 1024
HEADS = 16
D = 64
SCALE = D ** -0.5
G = 4            # head groups (tensor-parallel degree)
HPG = 4          # heads per group (local heads per core)
DL = HPG * D     # 256 local inner dim
NQK = 2 * DL     # 512 local q+k columns
KC = C // 128    # 8 contraction chunks
NB = N // 128    # 16 n-blocks
IBS = 512        # i-block size
IB = N // IBS    # 4 i-blocks
JBL = N // 128   # 16 j-blocks

EXPJ = 2         # j-blocks per exp op (psum scores tile = [128, EXPJ, 512])


def build_graph(expj=EXPJ, loop_r=None, exp_half=False):
    import contextlib

    nc = bacc.Bacc("TRN2")
    xT_ext = nc.declare_dram_parameter("xT", [C, N], BF, isOutput=False)
    wqk_ext = nc.declare_dram_parameter("wqk", [C, NQK], BF, isOutput=False)
    wv_ext = nc.declare_dram_parameter("wv", [C, DL], BF, isOutput=False)
    wp_ext = nc.declare_dram_parameter("wp", [DL, C], BF, isOutput=False)
    y_ext = nc.declare_dram_parameter("y", [N, C], F32, isOutput=True)

    Exp = mybir.ActivationFunctionType.Exp

    with tile.TileContext(nc) as tc:
        with (
            tc.tile_pool(name="const", bufs=1) as cpool,
            tc.tile_pool(name="probs", bufs=3) as probs_pool,
            tc.tile_pool(name="veca", bufs=2) as veca_pool,
            tc.tile_pool(name="recip1", bufs=1) as recip_pool,
            tc.tile_pool(name="obuf", bufs=2) as ob_pool,
            tc.tile_pool(name="yout", bufs=2) as y_pool,
            (tc.For_i(0, loop_r, 1) if loop_r else contextlib.nullcontext()),
        ):
            xT_sb = cpool.tile([128, KC, N], BF, tag="xT")
            wqk_sb = cpool.tile([128, KC, NQK], BF, tag="wqk")
            wv_sb = cpool.tile([128, KC, DL], BF, tag="wv")
            wp_sb = cpool.tile([128, 2, C], BF, tag="wp")
            qkT_sb = cpool.tile([128, 4, N], BF, tag="qkT")
            v_sb = cpool.tile([128, NB, HPG, D + 1], BF, tag="v")
            outT_sb = cpool.tile([128, 2, N], BF, tag="outT")

            for k in range(KC):
                nc.sync.dma_start(xT_sb[:, k, :], xT_ext[k * 128:(k + 1) * 128, :])
                nc.sync.dma_start(wqk_sb[:, k, :], wqk_ext[k * 128:(k + 1) * 128, :])
                nc.sync.dma_start(wv_sb[:, k, :], wv_ext[k * 128:(k + 1) * 128, :])
            for kc in range(2):
                nc.sync.dma_start(wp_sb[:, kc, :], wp_ext[kc * 128:(kc + 1) * 128, :])

            # ones column for the v-augmented denominator trick
            for nb in range(NB):
                nc.vector.memset(v_sb[:, nb, :, D:D + 1], 1.0)

            ss_sizes = []
            j = 0
            while j < JBL:
                n = min(expj, JBL - j)
                ss_sizes.append((j, n))
                j += n

            with (
                tc.tile_pool(name="ps_qk", bufs=2, space="PSUM") as psA,
                tc.tile_pool(name="ps_s", bufs=2, space="PSUM") as pss_pool,
                tc.tile_pool(name="ps_o", bufs=2, space="PSUM") as pso_pool,
            ):
                def emit_qk(mi, blocks=range(4)):
                    # qk^T: out [m-block 128, n 512] = wqk_slice^T @ xT
                    for nb4 in blocks:
                        ps = psA.tile([128, 512], F32, tag="psqk")
                        for k in range(KC):
                            nc.tensor.matmul(
                                ps[:],
                                wqk_sb[:, k, mi * 128:(mi + 1) * 128],
                                xT_sb[:, k, nb4 * 512:(nb4 + 1) * 512],
                                start=(k == 0),
                                stop=(k == KC - 1),
                            )
                        nc.vector.tensor_copy(
                            qkT_sb[:, mi, nb4 * 512:(nb4 + 1) * 512], ps[:]
                        )

                def emit_v():
                    # v natural: out [n-block 128, 256] = xT_slice^T @ wv
                    for nb in range(NB):
                        psv = psA.tile([128, DL], F32, tag="psqk")
                        for k in range(KC):
                            nc.tensor.matmul(
                                psv[:],
                                xT_sb[:, k, nb * 128:(nb + 1) * 128],
                                wv_sb[:, k, :],
                                start=(k == 0),
                                stop=(k == KC - 1),
                            )
                        nc.vector.tensor_copy(
                            v_sb[:, nb, :, 0:D],
                            psv[:].rearrange("p (h d) -> p h d", d=D),
                        )

                probs_tiles = {}

                def emit_scores_superstep(ib, hp, jb):
                    # heads (2hp, 2hp+1): row-group-paired score matmuls.
                    # Head parity selects PE row groups (lhsT partition base 0
                    # vs 64), so adjacent per-head matmuls run concurrently.
                    i0 = ib * IBS
                    probs = probs_tiles[(ib, hp)]
                    pss = pss_pool.tile([128, 2, IBS], F32, tag="pss")
                    for hs in range(2):
                        hb = hs * 64
                        nc.tensor.matmul(
                            pss[:, hs, :],
                            qkT_sb[hb:hb + 64, 2 + hp,
                                   jb * 128:(jb + 1) * 128],
                            qkT_sb[hb:hb + 64, hp, i0:i0 + IBS],
                            start=True,
                            stop=True,
                        )
                    if not exp_half or jb % 2 == 0:
                        nc.scalar.activation(probs[:, jb, :, :], pss[:], Exp)

                def alloc_probs(ib, hp):
                    probs_t = probs_pool.tile([128, JBL, 2, IBS], BF, tag="probs")
                    probs_tiles[(ib, hp)] = probs_t

                def emit_scores_pair(ib, hp):
                    alloc_probs(ib, hp)
                    for jb in range(JBL):
                        emit_scores_superstep(ib, hp, jb)

                def emit_norm(ib, h, pso):
                    # outT[hb:hb+64, hp, i] = pso[0:64] / pso[64]. Copy the
                    # PSUM accumulator to SBUF first so its bank frees for the
                    # next unit's av matmuls ~3us earlier than the full
                    # recip/broadcast/mul chain would allow.
                    i0 = ib * IBS
                    hb = (h % 2) * 64
                    hp = h // 2
                    ob = ob_pool.tile([65, IBS], F32, tag="ob")
                    nc.vector.tensor_copy(ob[:], pso[:])
                    recip = recip_pool.tile([1, IBS], F32, tag="recip")
                    nc.vector.reciprocal(recip[0:1, :], ob[64:65, :])
                    rep = veca_pool.tile([64, IBS], F32, tag="rep")
                    nc.gpsimd.partition_broadcast(rep[:], recip[0:1, :])
                    nc.vector.tensor_mul(
                        outT_sb[hb:hb + 64, hp, i0:i0 + IBS],
                        ob[0:64, :],
                        rep[:],
                    )

                ysb_tiles = {}

                def emit_proj_piece(nb, cb):
                    if cb == 0:
                        ysb_t = y_pool.tile([128, C], F32, tag="ysb")
                        ysb_tiles[nb] = ysb_t
                    ysb = ysb_tiles[nb]
                    psy = psA.tile([128, 512], F32, tag="psqk")
                    for kc in range(2):
                        nc.tensor.matmul(
                            psy[:],
                            outT_sb[:, kc, nb * 128:(nb + 1) * 128],
                            wp_sb[:, kc, cb * 512:(cb + 1) * 512],
                            start=(kc == 0),
                            stop=(kc == 1),
                        )
                    nc.vector.tensor_copy(ysb[:, cb * 512:(cb + 1) * 512], psy[:])
                    if cb == 1:
                        nc.sync.dma_start(y_ext[nb * 128:(nb + 1) * 128, :], ysb[:])
                        ysb_tiles.pop(nb)

                # phase 1 interleaved with the start of attention: compute k01
                # and q01 first so head 0/1 softmax starts ~35us earlier; v and
                # the other head-pair's qk fill the PE while ACT churns exps.
                # ramp: score superstep jb only needs k column-block jb//4,
                # so interleave the k01 blocks with the first unit's scores —
                # the exp stream starts after ~2 qk blocks instead of 5.
                emit_qk(2, blocks=(0,))  # k heads 0,1, j-blocks 0-3
                emit_qk(0, blocks=(0,))  # q heads 0,1, i-block 0
                alloc_probs(0, 0)
                for jb in range(4):
                    emit_scores_superstep(0, 0, jb)
                for nb4 in (1, 2, 3):
                    emit_qk(2, blocks=(nb4,))
                    for jb in range(4 * nb4, 4 * nb4 + 4):
                        emit_scores_superstep(0, 0, jb)
                emit_qk(0, blocks=(1, 2, 3))
                emit_qk(3)              # k heads 2,3
                emit_qk(1)              # q heads 2,3
                emit_scores_pair(0, 1)
                emit_v()
                # steady state, interleaved at j-block granularity so ScalarE
                # always has a fresh score superstep to exp: per unit u emit
                # [scores(u+2, jb), av(u, even, jb), av(u, odd, jb), proj piece]
                units = [(ib, hp) for ib in range(IB) for hp in range(2)]
                pending_proj = []
                for idx, (ib, hp) in enumerate(units):
                    nxt = units[idx + 2] if idx + 2 < len(units) else None
                    if nxt is not None:
                        probs_n = probs_pool.tile([128, JBL, 2, IBS], BF, tag="probs")
                        probs_tiles[nxt] = probs_n
                    probs_c = probs_tiles.pop((ib, hp))
                    pso_e = pso_pool.tile([65, IBS], F32, tag="pso")
                    pso_o = pso_pool.tile([65, IBS], F32, tag="pso")
                    for jb in range(JBL):
                        if nxt is not None:
                            emit_scores_superstep(nxt[0], nxt[1], jb)
                        nc.tensor.matmul(
                            pso_e[:], v_sb[:, jb, 2 * hp, :],
                            probs_c[:, jb, 0, :],
                            start=(jb == 0), stop=(jb == JBL - 1),
                        )
                        nc.tensor.matmul(
                            pso_o[:], v_sb[:, jb, 2 * hp + 1, :],
                            probs_c[:, jb, 1, :],
                            start=(jb == 0), stop=(jb == JBL - 1),
                        )
                        if pending_proj and jb % 2 == 1:
                            emit_proj_piece(*pending_proj.pop(0))
                    emit_norm(ib, 2 * hp, pso_e)
                    emit_norm(ib, 2 * hp + 1, pso_o)
                    if hp == 1:
                        pending_proj.extend(
                            (nb, cb)
                            for nb in range(ib * 4, ib * 4 + 4)
                            for cb in range(2)
                        )
                while pending_proj:
                    emit_proj_piece(*pending_proj.pop(0))

    nc.finalize()
    return nc


_GRAPH = None


def _get_graph():
    global _GRAPH
    if _GRAPH is None:
        _GRAPH = build_graph()
    return _GRAPH


def make_in_maps(x, w_qkv, w_proj):
    bf = ml_dtypes.bfloat16
    in_maps = []
    for core in range(8):
        b = core // G
        g = core % G
        wq = w_qkv[:, g * DL:(g + 1) * DL] * SCALE
        wk = w_qkv[:, C + g * DL:C + (g + 1) * DL]
        in_maps.append({
            "xT": np.ascontiguousarray(x[b].T).astype(bf),
            "wqk": np.concatenate([wq, wk], axis=1).astype(bf),
            "wv": np.ascontiguousarray(w_qkv[:, 2 * C + g * DL:2 * C + (g + 1) * DL]).astype(bf),
            "wp": np.ascontiguousarray(w_proj[g * DL:(g + 1) * DL, :]).astype(bf),
        })
    return in_maps


def gather_output(results, b_proj):
    y = np.zeros((B, N, C), np.float32)
    for core in range(8):
        y[core // G] += np.asarray(results[core]["y"], np.float32)
    y += np.asarray(b_proj, np.float32)[None, None, :]
    return y


def kernel(x, w_qkv, w_proj, b_proj):
    import time as _time

    nc = _get_graph()
    in_maps = make_in_maps(np.asarray(x, np.float32), np.asarray(w_qkv, np.float32),
                           np.asarray(w_proj, np.float32))
    last_err = None
    for attempt in range(3):
        try:
            res = run_bass_kernel_spmd(nc, in_maps, core_ids=list(range(8)))
            return gather_output(res.results, np.asarray(b_proj, np.float32))
        except Exception as e:  # transient NRT device errors: retry
            last_err = e
            _time.sleep(5.0 * (attempt + 1))
    raise last_err
